# revision 24
# baseline (speedup 1.0000x reference)
"""Trainium2 Bass kernel for nn_CosineSimilarity (segment_reduce).

reference semantics:
  x1, x2: [512, 256, 256] f32. Flatten each sample to 65536 elements.
  cos[i] = dot(a_i, b_i) / max(|a_i|*|b_i|, 1e-8)        (512 values)
  out[g] = mean(cos[8g:8g+8])                             ([64] f32)

Distribution: data-parallel over 8 NeuronCores, 64 samples (8 groups)
per core, no cross-core communication.

Per-core layout: sample s is split across 2 SBUF partitions (p = 2s+h,
h in {0,1}; 32768 elements per partition), streamed in chunks of
[128, f] f32 per input. Transfers ride the TWO hardware DGE rings (SP
and Activation), each chunk's x1/x2 halves on opposite rings,
alternating per chunk so both rings carry exactly half the bytes.
Trace evidence: the two HW rings sustain ~424 GB/s aggregate with
uniform occupancy across all 16 DMA queues (documented 358 GB/s/core
is conservative); adding the Pool SWDGE as a third bulk ring adds NO
bandwidth and systematically overloads DMA queue 15 by 3-9 us, gating
the stream end -- Pool carries only the tiny pm/gm constants.
Per chunk:
  DVE: scalar_tensor_tensor (a*1.0)*b, fp32 accum -> sum(a*b)
  ACT: activation(Square) with accum  -> sum(a*a), plus the first
       5/8 of each chunk's sum(b*b); the remaining 3/8 runs on DVE.
       The column-wise split keeps both engines at a uniform ~62% of
       the stream cadence (ACT alone at 2 squares/chunk was 103% of
       the 2-ring DMA cadence and added ~6 us of backlogged tail, and
       an odd/even whole-chunk split left DVE lumpy, which stalled the
       ACT ring's refill triggers on slot-reuse waits).
  PE : [128x64] pair-matrix matmul accumulates the chunk's
       (dot, s1, s2) partials straight into PSUM (start/stop flags),
       so no reduction work remains after the last byte lands.
The chunk schedule tapers early (...4096x6, 2048x3, 1024, 512, 512):
DVE enters the tail owing one full chunk's work (queueing lag), so
shrinking the chunks BEFORE the tail cuts the post-stream drain (a
queueing model over measured rates picked this schedule: ~2.1 us
drain vs 3.4 us for a 4096-heavy taper). (A 1536-col tail chunk
computed WRONG results on hardware while passing CoreSim - stick to
power-of-two chunk sizes.) Epilogue: copy PSUM->SBUF, prod = s1*s2, den =
Sqrt(prod + EPS^2) on ACT (bias tile fuses the eps clamp:
max(sqrt(x), eps) == sqrt(x + eps^2) to fp32 rounding), DVE
reciprocal, cos = dot*rec, then a [64x8] matmul (entries 1/8) for the
group means. (DVE stt cannot divide or read two PSUM operands; ACT
Rsqrt/Reciprocal are blocked for accuracy.)

Measured: v1 baseline (SP HWDGE + Pool SWDGE, both-squares-on-ACT,
end reduces): 100.6k ns clean / 115.7k noisy. Final design: 97.9k ns
clean (8.7 us fixed ramp + 79.0 us dense stream at ~424 GB/s + ~5 us
tail chains/epilogue + ~1.3 us out store + ~2.7 us NTFF). Each tail
chunk pays a ~1.1 us cross-engine stt/square/matmul chain, but the
chains pipeline against tail-chunk arrivals, so taper variants agree
within ~0.2 us. The ~8.7 us pre-stream ramp is fixed launch overhead
(invariant under ring choice, priming order, and first-transfer
descriptor count). Host co-tenancy adds 0-35 us of HBM-contention
jitter run to run; judge changes by trace structure, not single runs.
"""

import sys

if "/opt/trn_rl_repo" not in sys.path:
    sys.path.insert(0, "/opt/trn_rl_repo")

from contextlib import ExitStack

import numpy as np

import concourse.bacc as bacc
import concourse.bass as bass
import concourse.tile as tile
from concourse import mybir
from concourse.bass_utils import run_bass_kernel_spmd

N_CORES = 8
N_SAMPLES = 512
SAMPLE_LEN = 256 * 256          # 65536
GROUP = 8                       # segment length n
PER_CORE = N_SAMPLES // N_CORES  # 64 samples
HALF = SAMPLE_LEN // 2          # 32768 elements per partition
P = 128                         # SBUF partitions
BUFS = 5                        # stream buffer depth per input
CHUNKS = [4096] * 6 + [2048] * 3 + [1024, 512, 512]   # sum = HALF
NCH = len(CHUNKS)
EPS = 1e-8

FP32 = mybir.dt.float32
BF16 = mybir.dt.bfloat16

# --- DMA ring assignment -------------------------------------------------
# Rings: 0 = SP HWDGE, 1 = ACT HWDGE (Pool SWDGE carries only pm/gm).
# Bulk Pool-SWDGE traffic systematically overloads DMA queue 15 by
# 3-9 us (seen in every 3-ring run) and gates the stream end, while two
# HW rings alone reach the same ~424 GB/s aggregate with perfectly
# uniform queue occupancy. Each chunk's a/b halves ride opposite rings
# and alternate per chunk, so both rings carry exactly HALF cols.
A_RING = [c % 2 for c in range(NCH)]
B_RING = [1 - (c % 2) for c in range(NCH)]
# b-square column split per chunk: first B2A cols on ACT, rest on DVE, so
# both engines run ~62% of the stream cadence uniformly (no lumpy chunks).
B2A = [(5 * f) // 8 for f in CHUNKS]


def _build_program() -> bacc.Bacc:
    nc = bacc.Bacc("TRN2", target_bir_lowering=False, debug=False,
                   enable_asserts=False)

    x1 = nc.dram_tensor("x1", [PER_CORE, SAMPLE_LEN], FP32,
                        kind="ExternalInput").ap()
    x2 = nc.dram_tensor("x2", [PER_CORE, SAMPLE_LEN], FP32,
                        kind="ExternalInput").ap()
    pairmat = nc.dram_tensor("pairmat", [P, PER_CORE], FP32,
                             kind="ExternalInput").ap()
    groupmat = nc.dram_tensor("groupmat", [PER_CORE, GROUP], FP32,
                              kind="ExternalInput").ap()
    out = nc.dram_tensor("out", [GROUP, 1], FP32, kind="ExternalOutput").ap()

    # [64, 65536] -> [(64 s, 2 h) = 128, 32768]
    x1v = x1.rearrange("s (h r) -> (s h) r", h=2)
    x2v = x2.rearrange("s (h r) -> (s h) r", h=2)

    offsets = []
    o = 0
    for f in CHUNKS:
        offsets.append(o)
        o += f

    dma_engines = None  # set inside context

    with tile.TileContext(nc) as tc, ExitStack() as ctx:
        dma_engines = [nc.sync, nc.scalar, nc.gpsimd]

        const_pool = ctx.enter_context(tc.tile_pool(name="const", bufs=1))
        stat_pool = ctx.enter_context(tc.tile_pool(name="stat", bufs=1))
        stc_pool = ctx.enter_context(tc.tile_pool(name="stc", bufs=3))
        xa_pool = ctx.enter_context(tc.tile_pool(name="xa", bufs=BUFS))
        xb_pool = ctx.enter_context(tc.tile_pool(name="xb", bufs=BUFS))
        scr_pool = ctx.enter_context(tc.tile_pool(name="scr", bufs=1))
        psum_pool = ctx.enter_context(
            tc.tile_pool(name="psum", bufs=1, space="PSUM"))

        # eps^2 bias tile for the fused sqrt clamp; memset is Pool-side and
        # nearly free, and must precede the ACT warm-up that reads it.
        epsb = stat_pool.tile([PER_CORE, 1], FP32, tag="epsb")
        nc.gpsimd.memset(epsb[:], EPS * EPS)

        a_tiles: list = [None] * NCH
        b_tiles: list = [None] * NCH

        def load_chunk(c: int):
            f, offs = CHUNKS[c], offsets[c]
            a_tiles[c] = xa_pool.tile([P, f], FP32, tag="a", name=f"a{c}")
            dma_engines[A_RING[c]].dma_start(
                out=a_tiles[c][:], in_=x1v[:, offs:offs + f])
            b_tiles[c] = xb_pool.tile([P, f], FP32, tag="b", name=f"b{c}")
            dma_engines[B_RING[c]].dma_start(
                out=b_tiles[c][:], in_=x2v[:, offs:offs + f])

        # Prime all three rings with the first BUFS chunks before any
        # compute lands on SP/ACT/Pool sequencers.
        for c in range(min(BUFS, NCH)):
            load_chunk(c)

        # Constants ride the Pool SWDGE behind the primed chunks; pm is
        # needed by the first PSUM matmul (~25 us in), gm by the epilogue.
        pm = const_pool.tile([P, PER_CORE], FP32, tag="pm")
        nc.gpsimd.dma_start(out=pm[:], in_=pairmat[:])
        gm = const_pool.tile([PER_CORE, GROUP], FP32, tag="gm")
        nc.gpsimd.dma_start(out=gm[:], in_=groupmat[:])

        # Touch Sqrt AFTER the ring-priming triggers: the ACT table set
        # (sqrt_and_others, which also holds square) loads during the DMA
        # stream instead of delaying the b-ring or the epilogue.
        warm = stat_pool.tile([1, 1], FP32, tag="warm")
        nc.scalar.activation(warm[:], epsb[:1, :],
                             func=mybir.ActivationFunctionType.Sqrt)

        # PSUM accumulator for the [64, 4] (dot, s1, s2a, s2d) totals.
        psA = psum_pool.tile([PER_CORE, 4], FP32, tag="psA")

        for c, f in enumerate(CHUNKS):
            a, b = a_tiles[c], b_tiles[c]

            # Per-chunk stats tile: dot partial in col 0 (DVE), s1 in col 1
            # (ACT), s2 split col 2 (ACT part) + col 3 (DVE part). Folded
            # into PSUM by PE below; the epilogue re-adds cols 2+3.
            stc = stc_pool.tile([P, 4], FP32, tag="stc")

            # NOTE: native InstTensorTensorReduce crashes the device on this
            # firmware; scalar_tensor_tensor is the working fused
            # multiply+accumulate on DVE: out=(a*1.0)*b, accum=sum(out).
            # Scratch tiles are bf16 (accumulator stays fp32 internally)
            # and per-engine-stream tagged so slots never cross engines.
            so = scr_pool.tile([P, f], BF16, tag="scr_dve")
            nc.vector.scalar_tensor_tensor(
                out=so[:], in0=a[:], scalar=1.0, in1=b[:],
                op0=mybir.AluOpType.mult, op1=mybir.AluOpType.mult,
                accum_out=stc[:, 0:1])

            sa = scr_pool.tile([P, f], BF16, tag="scr_a")
            nc.scalar.activation(
                out=sa[:], in_=a[:], func=mybir.ActivationFunctionType.Square,
                accum_out=stc[:, 1:2])

            wa = B2A[c]
            sba = scr_pool.tile([P, wa], BF16, tag="scr_b_act")
            nc.scalar.activation(
                out=sba[:], in_=b[:, :wa],
                func=mybir.ActivationFunctionType.Square,
                accum_out=stc[:, 2:3])
            sbd = scr_pool.tile([P, f - wa], BF16, tag="scr_b_dve")
            nc.vector.scalar_tensor_tensor(
                out=sbd[:], in0=b[:, wa:], scalar=1.0, in1=b[:, wa:],
                op0=mybir.AluOpType.mult, op1=mybir.AluOpType.mult,
                accum_out=stc[:, 3:4])

            # Fold partition halves into per-sample partials, accumulating
            # across chunks in PSUM: psA += pairmat.T @ stc.
            nc.tensor.matmul(psA[:], pm[:], stc[:],
                             start=(c == 0), stop=(c == NCH - 1))

            # Refill triggers AFTER this chunk's consumers so a ring
            # trigger never waits (on slot reuse) for a compute op that
            # sits behind it in the same engine's program order.
            nx = c + BUFS
            if nx < NCH:
                load_chunk(nx)

        # Epilogue on [64, 1] tiles. (DVE stt can't divide or read two PSUM
        # operands per the ISA checks, so: copy out of PSUM, reciprocal.)
        st = stat_pool.tile([PER_CORE, 4], FP32, tag="st")
        nc.vector.tensor_copy(st[:], psA[:])
        # prod = (s2a + s2d) * s1 in one DVE op (scalar slot is a [P,1] AP)
        prod = stat_pool.tile([PER_CORE, 1], FP32, tag="prod")
        nc.vector.scalar_tensor_tensor(
            out=prod[:], in0=st[:, 2:3], scalar=st[:, 3:4], in1=st[:, 1:2],
            op0=mybir.AluOpType.add, op1=mybir.AluOpType.mult)
        # max(sqrt(x), EPS) == sqrt(x + EPS^2) within fp32 rounding.
        den = stat_pool.tile([PER_CORE, 1], FP32, tag="den")
        nc.scalar.activation(den[:], prod[:],
                             func=mybir.ActivationFunctionType.Sqrt,
                             bias=epsb[:])
        rec = stat_pool.tile([PER_CORE, 1], FP32, tag="rec")
        nc.vector.reciprocal(rec[:], den[:])
        cos = stat_pool.tile([PER_CORE, 1], FP32, tag="cos")
        nc.vector.tensor_mul(cos[:], st[:, 0:1], rec[:])

        # group means: [8, 1] = groupmat.T @ cos (groupmat entries are 1/8)
        psB = psum_pool.tile([GROUP, 1], FP32, tag="psB")
        nc.tensor.matmul(psB[:], gm[:], cos[:], start=True, stop=True)
        res = stat_pool.tile([GROUP, 1], FP32, tag="res")
        nc.vector.tensor_copy(res[:], psB[:])
        nc.sync.dma_start(out=out[:], in_=res[:])

    nc.compile()
    return nc


_PROGRAM: bacc.Bacc | None = None


def _get_program() -> bacc.Bacc:
    global _PROGRAM
    if _PROGRAM is None:
        _PROGRAM = _build_program()
    return _PROGRAM


def _constants() -> tuple[np.ndarray, np.ndarray]:
    pm = np.zeros((P, PER_CORE), dtype=np.float32)
    pm[np.arange(P), np.arange(P) // 2] = 1.0
    gm = np.zeros((PER_CORE, GROUP), dtype=np.float32)
    gm[np.arange(PER_CORE), np.arange(PER_CORE) // GROUP] = 1.0 / GROUP
    return pm, gm


def _run(in_maps, trace: bool = False, **kw):
    nc = _get_program()
    return run_bass_kernel_spmd(nc, in_maps, list(range(N_CORES)),
                                trace=trace, **kw)


def _make_in_maps(x1: np.ndarray, x2: np.ndarray) -> list[dict]:
    pm, gm = _constants()
    s1 = x1.reshape(N_CORES, PER_CORE, SAMPLE_LEN)
    s2 = x2.reshape(N_CORES, PER_CORE, SAMPLE_LEN)
    return [
        {"x1": s1[k], "x2": s2[k], "pairmat": pm, "groupmat": gm}
        for k in range(N_CORES)
    ]


def kernel(x1, x2, n):
    x1 = np.ascontiguousarray(np.asarray(x1, dtype=np.float32))
    x2 = np.ascontiguousarray(np.asarray(x2, dtype=np.float32))
    n = int(np.asarray(n))
    assert n == GROUP, f"kernel compiled for n={GROUP}, got {n}"
    assert x1.shape == (N_SAMPLES, 256, 256) and x2.shape == x1.shape

    in_maps = _make_in_maps(x1, x2)
    # The axon-tunneled devices occasionally report a transient
    # NRT_EXEC_UNIT_UNRECOVERABLE from a previous tenant; re-running
    # (after a backend reset) recovers.
    last_err = None
    for attempt in range(3):
        try:
            res = _run(in_maps)
            break
        except Exception as e:  # noqa: BLE001 - jax runtime errors
            last_err = e
            import time

            time.sleep(5 * (attempt + 1))
            try:
                import jax

                jax.clear_backends()
            except Exception:
                pass
    else:
        raise last_err

    return np.concatenate(
        [res.results[k]["out"].reshape(GROUP) for k in range(N_CORES)]
    ).astype(np.float32)
